# revision 1
# baseline (speedup 1.0000x reference)
"""Trainium2 Bass kernel for nn_BinaryLinear (binary-weight linear + BatchNorm + sign).

Computation (reference):
    bw    = sign(W)                     # [O, I], entries in {-1, 0, +1}
    alpha = mean(|W|, axis=1)           # [O]
    y     = x @ (bw * alpha).T          # [B, O]
    out   = sign((y - mu_b) / sqrt(var_b + eps) * gamma + beta)   # batch stats

Strategy (8 NeuronCores, column-sharded):
  * Each core owns O/8 = 512 output columns; BN batch stats are then fully
    local to a core (full batch for its columns) -> no collectives.
  * alpha is factored out of the matmul: s = x @ bw.T is computed on the PE
    with bw exact in bf16; x is split as x = hi + lo (two bf16 matmuls
    accumulated in fp32 PSUM) which recovers ~16 mantissa bits of x.
  * Layout is transposed on host: s.T[o, b] so that o sits on SBUF
    partitions. BN stats are per-partition reductions along the free dim
    (BN_STATS/BN_AGGR), and the final affine+sign is a single ScalarE
    activation with per-partition scale/bias.
  * Host precomputes bw, the per-column BN coefficient inputs, and the
    hi/lo split; host transposes/assembles the output.
"""

import os
from contextlib import ExitStack

import ml_dtypes
import numpy as np

import concourse.bacc as bacc
import concourse.bass as bass
import concourse.mybir as mybir
import concourse.tile as tile
from concourse.bass_utils import run_bass_kernel_spmd

BF16 = ml_dtypes.bfloat16
BN_EPS = 1e-5

N_CORES = 8
B_FULL, IN_F, OUT_F = 8192, 4096, 4096

LAST_RESULTS = None  # BassKernelResults of the most recent device run


def build_nc(B, I, OSH, CH=512, xbufs=4, simple_tail=False):
    """Build + compile the per-core Bass program.

    B: batch (free dim of s.T), I: contraction, OSH: output columns per core,
    CH: batch chunk (<=512, PSUM bank / bn_stats limit). simple_tail may only
    be set when gamma > 0 and beta == 0 (sign(BN(y)) == sign(s - mean_s)).
    """
    NOT = OSH // 128          # o-tiles (PSUM partition groups)
    NT = I // 128             # i-tiles (contraction)
    NCH = B // CH             # batch chunks
    f32 = mybir.dt.float32
    bf16 = mybir.dt.bfloat16

    nc = bacc.Bacc("TRN2", target_bir_lowering=False, debug=False)
    # x tiles packed in i-tile PAIRS: one DMA delivers hi|lo chunks for two
    # consecutive i-tiles (512 KB) -> half the DMA/wait count on the PE.
    xtp_d = nc.dram_tensor(
        "xtp", [NT // 2, NCH, 128, 4 * CH], bf16, kind="ExternalInput"
    )
    bwt_d = nc.dram_tensor("bwt", [NT, 128, OSH], bf16, kind="ExternalInput")
    coef_d = nc.dram_tensor("coef", [128, 4 * NOT], f32, kind="ExternalInput")
    out_d = nc.dram_tensor("out", [OSH, B], bf16, kind="ExternalOutput")
    SGW = min(B, 2048)        # sign-pass slab width
    NSG = B // SGW

    with tile.TileContext(nc) as tc, ExitStack() as ctx:
        bw_pool = ctx.enter_context(tc.tile_pool(name="bw", bufs=NT))
        x_pool = ctx.enter_context(tc.tile_pool(name="x", bufs=xbufs))
        # bw/coef loads go on GpSimd (SWDGE) so the SP HWDGE ring starts
        # streaming x tiles immediately -> first matmul fires early.
        y_pool = ctx.enter_context(tc.tile_pool(name="y", bufs=1))
        ps_pool = ctx.enter_context(
            tc.tile_pool(name="ps", bufs=8, space=bass.MemorySpace.PSUM)
        )
        st_pool = ctx.enter_context(tc.tile_pool(name="st", bufs=1))
        sg_pool = ctx.enter_context(tc.tile_pool(name="sg", bufs=4))
        sm_pool = ctx.enter_context(tc.tile_pool(name="sm", bufs=NOT))

        # PE warm-up: the HAM clock gate holds the PE at 1.2 GHz until it has
        # been busy ~3.4us. Burn dummy matmuls during the initial DMA wait so
        # the real matmul stream starts at 2.4 GHz.
        wl = sm_pool.tile([128, 64], bf16)
        wr = sm_pool.tile([128, 64], bf16)
        nc.vector.memset(wl[:], 0.0)
        nc.vector.memset(wr[:], 0.0)
        wp = ps_pool.tile([128, CH], f32, name="wups", tag="ps")
        for _ in range(40):
            nc.tensor.matmul(wp[0:64, 0:64], wl[:], wr[:], start=True, stop=True)

        bw_tiles = [None] * NT
        # First two weight tiles ride the ACT HWDGE ring (ahead of its x-DMA
        # work) so the SP ring's first transfer is x tile 0; the rest of the
        # weights go via GpSimd SWDGE off the critical path.
        for t in range(NT):
            bt = bw_pool.tile([128, OSH], bf16, name=f"bt{t}", tag="bt")
            if t < 2:
                nc.scalar.dma_start(bt[:], bwt_d.ap()[t])
            else:
                nc.gpsimd.dma_start(bt[:], bwt_d.ap()[t])
            bw_tiles[t] = bt

        ct = sm_pool.tile([128, 4 * NOT], f32)
        nc.gpsimd.dma_start(ct[:], coef_d.ap())

        yt = [y_pool.tile([128, B], f32, name=f"yt{i}") for i in range(NOT)]
        stats = [st_pool.tile([128, 6 * NCH], f32, name=f"stats{i}") for i in range(NOT)]

        # Per-o-tile BN coefficients: with s-stats (mean_s, var_s) and host
        # precomputed p1=alpha^2, p2=alpha*gamma, p4=beta:
        #   inv = 1/sqrt(p1*var_s + eps);  A = p2*inv;  B = p4 - mean_s*A
        A_t, B_t, mv_t = [None] * NOT, [None] * NOT, [None] * NOT
        eps_t = sm_pool.tile([128, 1], f32)
        nc.vector.memset(eps_t[:], BN_EPS)

        def coef_chain(ot):
            mv = sm_pool.tile([128, 2], f32, name=f"mv{ot}", tag="mv")
            nc.vector.bn_aggr(mv[:], stats[ot][:])
            p1 = ct[:, ot : ot + 1]
            p2 = ct[:, NOT + ot : NOT + ot + 1]
            p4 = ct[:, 3 * NOT + ot : 3 * NOT + ot + 1]
            v = sm_pool.tile([128, 1], f32, name=f"v{ot}", tag="v")
            nc.vector.tensor_mul(v[:], mv[:, 1:2], p1)
            sd = sm_pool.tile([128, 1], f32, name=f"sd{ot}", tag="sd")
            nc.scalar.activation(
                sd[:], v[:], mybir.ActivationFunctionType.Sqrt, bias=eps_t[:]
            )
            inv = sm_pool.tile([128, 1], f32, name=f"inv{ot}", tag="inv")
            nc.vector.reciprocal(inv[:], sd[:])
            Ac = sm_pool.tile([128, 1], f32, name=f"Ac{ot}", tag="Ac")
            nc.vector.tensor_mul(Ac[:], p2, inv[:])
            mB = sm_pool.tile([128, 1], f32, name=f"mB{ot}", tag="mB")
            nc.vector.tensor_mul(mB[:], mv[:, 0:1], Ac[:])
            Bc = sm_pool.tile([128, 1], f32, name=f"Bc{ot}", tag="Bc")
            nc.vector.tensor_sub(Bc[:], p4, mB[:])
            A_t[ot], B_t[ot], mv_t[ot] = Ac, Bc, mv

        for c in range(NCH):
            if c == 1:
                # Preload the tail ACT LUTs (Sqrt, Sign) once the stream is
                # rolling: no ACT_TABLE_LOAD on the critical tail, and no
                # delay to the startup bw/x DMA issues on the ACT ring.
                wt = sm_pool.tile([128, 1], f32)
                nc.vector.memset(wt[:], 1.0)
                wt2 = sm_pool.tile([128, 1], f32)
                nc.scalar.activation(
                    wt2[:], wt[:], mybir.ActivationFunctionType.Sqrt
                )
                nc.scalar.activation(
                    wt2[:], wt[:], mybir.ActivationFunctionType.Sign
                )
            ps = [ps_pool.tile([128, CH], f32, name=f"ps{c}_{i}", tag="ps") for i in range(NOT)]
            for u in range(NT // 2):
                xt = x_pool.tile([128, 4 * CH], bf16)
                # alternate HWDGE rings (SP / ACT) so the x stream has 2x
                # DMA headroom over PE consumption
                dma_eng = nc.sync if u % 2 == 0 else nc.scalar
                dma_eng.dma_start(xt[:], xtp_d.ap()[u, c])
                for tt in range(2):
                    t = 2 * u + tt
                    base = tt * 2 * CH
                    for ot in range(NOT):
                        lhsT = bw_tiles[t][:, ot * 128 : (ot + 1) * 128]
                        nc.tensor.matmul(
                            ps[ot][:], lhsT, xt[:, base : base + CH],
                            start=(t == 0), stop=False,
                        )
                        nc.tensor.matmul(
                            ps[ot][:], lhsT, xt[:, base + CH : base + 2 * CH],
                            start=False, stop=(t == NT - 1),
                        )
            for ot in range(NOT):
                ysl = yt[ot][:, c * CH : (c + 1) * CH]
                # stats read PSUM directly so the tail-critical chain does not
                # wait on the ACT evacuation copy
                nc.vector.bn_stats(stats[ot][:, c * 6 : (c + 1) * 6], ps[ot][:])
                if c == NCH - 1:
                    # emit the coefficient chain right after this o-tile's
                    # final stats so the first sign slab starts ASAP; the
                    # final evacuation rides ACT so DVE reaches its sign
                    # slabs sooner
                    coef_chain(ot)
                    nc.scalar.copy(ysl, ps[ot][:])
                else:
                    nc.vector.tensor_copy(ysl, ps[ot][:])

        # Final affine+sign, split between ScalarE (one ACTIVATE(Sign) per
        # slab) and DVE so the serial tail drains on two engines at once.
        # With gamma>0 and beta==0 (the simple_tail case) the decision is just
        # s >= mean_s, so the DVE path is 2 ops; otherwise it is 3 ops.
        # Interleave ACT/DVE slabs in emission order: the shared sg-slot pool
        # recycles in allocation order, so grouping all DVE slabs last would
        # stall DVE behind ACT's DMA-outs.
        n_slabs = NOT * NSG
        if n_slabs >= 8:
            step = 2 if simple_tail else 3
            dve_ks = set(range(1, n_slabs, step))
        else:
            dve_ks = set()
        # GpSimd slabs measured 110us SLOWER overall: POOL shares SBUF ports
        # with DVE (exclusive lock), stalling DVE's own sign slabs.
        gps_ks = set()
        DVE_SLABS = {divmod(k, NOT)[::-1] for k in dve_ks}  # k=h*NOT+ot -> (ot,h)
        GPS_SLABS = {divmod(k, NOT)[::-1] for k in gps_ks}
        for h in range(NSG):
            for ot in range(NOT):
                ysl = yt[ot][:, h * SGW : (h + 1) * SGW]
                sg = sg_pool.tile([128, SGW], bf16, name=f"sg{ot}_{h}", tag="sg")
                if (ot, h) in GPS_SLABS:
                    # simple_tail only: sg = (s >= mean); sg = 2*sg - 1
                    nc.gpsimd.tensor_scalar(
                        sg[:], ysl, mv_t[ot][:, 0:1], None,
                        mybir.AluOpType.is_ge,
                    )
                    nc.gpsimd.tensor_scalar(
                        sg[:], sg[:], 2.0, 1.0,
                        mybir.AluOpType.mult, mybir.AluOpType.subtract,
                    )
                elif (ot, h) in DVE_SLABS:
                    if simple_tail:
                        # sg = (s >= mean) ; sg = 2*sg - 1
                        nc.vector.tensor_scalar(
                            sg[:], ysl, mv_t[ot][:, 0:1], None,
                            mybir.AluOpType.is_ge,
                        )
                        nc.vector.tensor_scalar(
                            sg[:], sg[:], 2.0, 1.0,
                            mybir.AluOpType.mult, mybir.AluOpType.subtract,
                        )
                    else:
                        # in-place: y' = y*A + B; sg = (y' >= 0); sg = 2*sg-1
                        nc.vector.tensor_scalar(
                            ysl, ysl, A_t[ot][:], B_t[ot][:],
                            mybir.AluOpType.mult, mybir.AluOpType.add,
                        )
                        nc.vector.tensor_scalar(
                            sg[:], ysl, 0.0, None, mybir.AluOpType.is_ge
                        )
                        nc.vector.tensor_scalar(
                            sg[:], sg[:], 2.0, 1.0,
                            mybir.AluOpType.mult, mybir.AluOpType.subtract,
                        )
                else:
                    nc.scalar.activation(
                        sg[:], ysl,
                        mybir.ActivationFunctionType.Sign,
                        bias=B_t[ot][:],
                        scale=A_t[ot][:],
                    )
                nc.sync.dma_start(
                    out_d.ap()[ot * 128 : (ot + 1) * 128, h * SGW : (h + 1) * SGW],
                    sg[:],
                )

    nc.compile()
    return nc


def prep_inputs(x, w, gamma, beta, n_cores=N_CORES, CH=512):
    """Host-side prep: hi/lo split + transpose of x, bw/coef shards per core."""
    B, I = x.shape
    O = w.shape[0]
    OSH = O // n_cores
    NT = I // 128
    NCH = B // CH

    x_hi = x.astype(BF16)
    x_lo = (x - x_hi.astype(np.float32)).astype(BF16)
    # [I, B] -> [NT, NCH, 128, CH] per pass, concat hi|lo on last axis
    def chunkify(xp):
        xt = np.ascontiguousarray(xp.T)           # [I, B]
        return (
            xt.reshape(NT, 128, NCH, CH).transpose(0, 2, 1, 3)
        )  # [NT, NCH, 128, CH]

    xtp = np.concatenate([chunkify(x_hi), chunkify(x_lo)], axis=3)
    # pack i-tile pairs: [NT, NCH, 128, 2CH] -> [NT//2, NCH, 128, 4CH]
    xtp = (
        xtp.reshape(NT // 2, 2, NCH, 128, 2 * CH)
        .transpose(0, 2, 3, 1, 4)
        .reshape(NT // 2, NCH, 128, 4 * CH)
    )
    xtp = np.ascontiguousarray(xtp)

    bw = np.sign(w).astype(np.float32)
    alpha = np.abs(w).mean(axis=1)                 # [O] f32
    p1 = alpha * alpha
    p2 = alpha * gamma
    p3 = alpha * alpha * gamma
    p4 = beta.astype(np.float32)

    in_maps = []
    for k in range(n_cores):
        sl = slice(k * OSH, (k + 1) * OSH)
        bwt = np.ascontiguousarray(bw[sl].T).reshape(NT, 128, OSH).astype(BF16)
        NOT = OSH // 128

        def per_tile(vec):
            return np.ascontiguousarray(vec[sl].reshape(NOT, 128).T)  # [128, NOT]

        coef = np.concatenate(
            [per_tile(p1), per_tile(p2), per_tile(p3), per_tile(p4)], axis=1
        ).astype(np.float32)
        in_maps.append({"xtp": xtp, "bwt": bwt, "coef": coef})
    return in_maps


_NC_CACHE = {}


def kernel(x, real_weight, gamma, beta):
    global LAST_RESULTS
    x = np.asarray(x, dtype=np.float32)
    w = np.asarray(real_weight, dtype=np.float32)
    gamma = np.asarray(gamma, dtype=np.float32)
    beta = np.asarray(beta, dtype=np.float32)
    B, I = x.shape
    O = w.shape[0]
    OSH = O // N_CORES
    CH = 512
    NCH = B // CH

    simple_tail = bool((gamma > 0).all() and (beta == 0).all())
    key = (B, I, OSH, CH, simple_tail)
    if key not in _NC_CACHE:
        _NC_CACHE[key] = build_nc(B, I, OSH, CH, simple_tail=simple_tail)
    nc = _NC_CACHE[key]

    in_maps = prep_inputs(x, w, gamma, beta, N_CORES, CH)
    trace = bool(int(os.environ.get("KERNEL_TRACE", "0")))
    res = run_bass_kernel_spmd(
        nc, in_maps, core_ids=list(range(N_CORES)), trace=trace
    )
    LAST_RESULTS = res

    out = np.empty((B, O), dtype=np.float32)
    for k in range(N_CORES):
        o = res.results[k]["out"]                  # [OSH, B] bf16
        out[:, k * OSH : (k + 1) * OSH] = o.T.astype(np.float32)
    return out



# revision 3
# speedup vs baseline: 1.3962x; 1.3962x over previous
"""Trainium2 Bass kernel for nn_BinaryLinear (binary-weight linear + BatchNorm + sign).

Computation (reference):
    bw    = sign(W)                     # [O, I], entries in {-1, 0, +1}
    alpha = mean(|W|, axis=1)           # [O]
    y     = x @ (bw * alpha).T          # [B, O]
    out   = sign((y - mu_b) / sqrt(var_b + eps) * gamma + beta)   # batch stats

Strategy (8 NeuronCores, column-sharded):
  * Each core owns O/8 = 512 output columns; BN batch stats are then fully
    local to a core (full batch for its columns) -> no collectives.
  * alpha is factored out of the matmul: s = x @ bw.T is computed on the PE
    in float32r (fp32 inputs truncated to ~fp22 internally): a SINGLE
    matmul pass recovers ~13.2 mantissa bits of x (HW-measured), keeping
    sign flips well under the rel-err gate while HALVING PE work vs the
    bf16 hi/lo split (f32r streams 1 row/cycle at free-dim 512; measured
    227 ns/MM warm vs 216 for bf16).
  * Weights (+-1 exact in any dtype) are shipped as int8 and expanded to
    f32 by SWDGE cast-DMA, so the startup weight load costs 2 MB of HBM
    traffic instead of 8 MB, which matters because chunk 0 already needs
    the full 8 MB x stream.
  * Layout is transposed on host: s.T[o, b] so that o sits on SBUF
    partitions. BN stats are per-partition reductions along the free dim
    (BN_STATS/BN_AGGR); the final affine+sign is ScalarE/DVE slabs writing
    int8 (+-1), halving the tail output DMA.
  * SBUF: w 64K/part + y 128K/part leaves ~16K: x pairs and sign slabs
    share one 3-slot 4K pool ring (tag "x").
"""

import os
from contextlib import ExitStack

import ml_dtypes
import numpy as np

import concourse.bacc as bacc
import concourse.bass as bass
import concourse.mybir as mybir
import concourse.tile as tile
from concourse.bass_utils import run_bass_kernel_spmd

BF16 = ml_dtypes.bfloat16
BN_EPS = 1e-5

N_CORES = 8
B_FULL, IN_F, OUT_F = 8192, 4096, 4096

LAST_RESULTS = None  # BassKernelResults of the most recent device run


def build_nc(B, I, OSH, CH=512, xbufs=3, simple_tail=False):
    """Build + compile the per-core Bass program.

    B: batch (free dim of s.T), I: contraction, OSH: output columns per core,
    CH: batch chunk (<=512, PSUM bank / bn_stats limit). simple_tail may only
    be set when gamma > 0 and beta == 0 (sign(BN(y)) == sign(s - mean_s)).
    """
    NOT = OSH // 128          # o-tiles (PSUM partition groups)
    NT = I // 128             # i-tiles (contraction)
    NCH = B // CH             # batch chunks
    f32 = mybir.dt.float32
    f32r = mybir.dt.float32r
    bf16 = mybir.dt.bfloat16
    i8 = mybir.dt.int8

    nc = bacc.Bacc("TRN2", target_bir_lowering=False, debug=False)
    # x tiles packed in i-tile PAIRS: one DMA delivers fp32 chunks for two
    # consecutive i-tiles (512 KB).
    xtp_d = nc.dram_tensor(
        "xtp", [NT // 2, NCH, 128, 2 * CH], f32r, kind="ExternalInput"
    )
    bwt_d = nc.dram_tensor("bwt", [NT, 128, OSH], i8, kind="ExternalInput")
    # first two i-tiles' weights also shipped raw f32: they ride the ACT
    # HWDGE ring (no cast on HWDGE) so the first matmul is not gated on
    # SWDGE spin-up.
    bwh_d = nc.dram_tensor("bwh", [2, 128, OSH], f32r, kind="ExternalInput")
    coef_d = nc.dram_tensor("coef", [128, 4 * NOT], f32, kind="ExternalInput")
    out_d = nc.dram_tensor("out", [OSH, B], i8, kind="ExternalOutput")
    SGW = min(B, 4096)        # sign-pass slab width (int8 slab = x slot size)
    NSG = B // SGW

    with tile.TileContext(nc) as tc, ExitStack() as ctx:
        bw_pool = ctx.enter_context(tc.tile_pool(name="bw", bufs=NT))
        x_pool = ctx.enter_context(tc.tile_pool(name="x", bufs=xbufs))
        y_pool = ctx.enter_context(tc.tile_pool(name="y", bufs=1))
        ps_pool = ctx.enter_context(
            tc.tile_pool(name="ps", bufs=8, space=bass.MemorySpace.PSUM)
        )
        st_pool = ctx.enter_context(tc.tile_pool(name="st", bufs=1))
        sm_pool = ctx.enter_context(tc.tile_pool(name="sm", bufs=NOT))

        # PE warm-up: the HAM clock gate holds the PE at 1.2 GHz until it has
        # been busy ~3.4us. Burn dummy matmuls during the initial DMA wait so
        # the real matmul stream starts at 2.4 GHz.
        wl = sm_pool.tile([128, 8], bf16)
        wr = sm_pool.tile([128, 8], bf16)
        nc.vector.memset(wl[:], 0.0)
        nc.vector.memset(wr[:], 0.0)
        wp = ps_pool.tile([128, CH], f32, name="wups", tag="ps")
        for _ in range(64):
            nc.tensor.matmul(wp[0:8, 0:8], wl[:], wr[:], start=True, stop=True)

        bw_tiles = [None] * NT
        for t in range(NT):
            bt = bw_pool.tile([128, OSH], f32r, name=f"bt{t}", tag="bt")
            if t < 2:
                nc.scalar.dma_start(bt[:], bwh_d.ap()[t])
            else:
                # int8 -> f32 cast during SWDGE DMA; values are +-1, exact.
                nc.gpsimd.dma_start(bt[:], bwt_d.ap()[t])
            bw_tiles[t] = bt

        ct = st_pool.tile([128, 4 * NOT], f32)
        nc.gpsimd.dma_start(ct[:], coef_d.ap())

        yt = [y_pool.tile([128, B], f32, name=f"yt{i}") for i in range(NOT)]
        stats = [st_pool.tile([128, 6 * NCH], f32, name=f"stats{i}") for i in range(NOT)]

        # Per-o-tile BN coefficients: with s-stats (mean_s, var_s) and host
        # precomputed p1=alpha^2, p2=alpha*gamma, p4=beta:
        #   inv = 1/sqrt(p1*var_s + eps);  A = p2*inv;  B = p4 - mean_s*A
        A_t, B_t, mv_t = [None] * NOT, [None] * NOT, [None] * NOT
        eps_t = sm_pool.tile([128, 1], f32)
        nc.vector.memset(eps_t[:], BN_EPS)

        def coef_chain(ot):
            mv = sm_pool.tile([128, 2], f32, name=f"mv{ot}", tag="mv")
            nc.vector.bn_aggr(mv[:], stats[ot][:])
            p1 = ct[:, ot : ot + 1]
            p2 = ct[:, NOT + ot : NOT + ot + 1]
            p4 = ct[:, 3 * NOT + ot : 3 * NOT + ot + 1]
            v = sm_pool.tile([128, 1], f32, name=f"v{ot}", tag="v")
            nc.vector.tensor_mul(v[:], mv[:, 1:2], p1)
            sd = sm_pool.tile([128, 1], f32, name=f"sd{ot}", tag="sd")
            nc.scalar.activation(
                sd[:], v[:], mybir.ActivationFunctionType.Sqrt, bias=eps_t[:]
            )
            inv = sm_pool.tile([128, 1], f32, name=f"inv{ot}", tag="inv")
            nc.vector.reciprocal(inv[:], sd[:])
            Ac = sm_pool.tile([128, 1], f32, name=f"Ac{ot}", tag="Ac")
            nc.vector.tensor_mul(Ac[:], p2, inv[:])
            mB = sm_pool.tile([128, 1], f32, name=f"mB{ot}", tag="mB")
            nc.vector.tensor_mul(mB[:], mv[:, 0:1], Ac[:])
            Bc = sm_pool.tile([128, 1], f32, name=f"Bc{ot}", tag="Bc")
            nc.vector.tensor_sub(Bc[:], p4, mB[:])
            A_t[ot], B_t[ot], mv_t[ot] = Ac, Bc, mv

        for c in range(NCH):
            if c == 1:
                # Preload the tail ACT LUTs (Sqrt, Sign) once the stream is
                # rolling: no ACT_TABLE_LOAD on the critical tail, and no
                # delay to the startup DMA issues on the ACT ring.
                wt = sm_pool.tile([128, 1], f32)
                nc.vector.memset(wt[:], 1.0)
                wt2 = sm_pool.tile([128, 1], f32)
                nc.scalar.activation(
                    wt2[:], wt[:], mybir.ActivationFunctionType.Sqrt
                )
                nc.scalar.activation(
                    wt2[:], wt[:], mybir.ActivationFunctionType.Sign
                )
            ps = [ps_pool.tile([128, CH], f32, name=f"ps{c}_{i}", tag="ps") for i in range(NOT)]
            for u in range(NT // 2):
                xt = x_pool.tile([128, 2 * CH], f32r, tag="x")
                # alternate HWDGE rings (SP / ACT) so the x stream has 2x
                # DMA headroom over PE consumption
                dma_eng = nc.sync if u % 2 == 0 else nc.scalar
                dma_eng.dma_start(xt[:], xtp_d.ap()[u, c])
                for tt in range(2):
                    t = 2 * u + tt
                    base = tt * CH
                    for ot in range(NOT):
                        lhsT = bw_tiles[t][:, ot * 128 : (ot + 1) * 128]
                        nc.tensor.matmul(
                            ps[ot][:], lhsT,
                            xt[:, base : base + CH],
                            start=(t == 0), stop=(t == NT - 1),
                        )
            for ot in range(NOT):
                ysl = yt[ot][:, c * CH : (c + 1) * CH]
                # stats read PSUM directly so the tail-critical chain does not
                # wait on the ACT evacuation copy
                nc.vector.bn_stats(stats[ot][:, c * 6 : (c + 1) * 6], ps[ot][:])
                if c == NCH - 1:
                    # emit the coefficient chain right after this o-tile's
                    # final stats so the first sign slab starts ASAP; the
                    # final evacuation rides ACT so DVE reaches its sign
                    # slabs sooner
                    coef_chain(ot)
                    nc.scalar.copy(ysl, ps[ot][:])
                else:
                    nc.vector.tensor_copy(ysl, ps[ot][:])

        # Final affine+sign, split between ScalarE (one ACTIVATE(Sign) per
        # slab, 1 op) and DVE (2 ops simple / 3 ops full) so the serial tail
        # drains on two engines at once. Slab outputs are int8 (+-1), which
        # halves the tail DMA-out bytes.
        n_slabs = NOT * NSG
        step = 3 if simple_tail else 4
        dve_ks = set(range(1, n_slabs, step)) if n_slabs >= 6 else set()
        DVE_SLABS = {divmod(k, NOT)[::-1] for k in dve_ks}  # k=h*NOT+ot -> (ot,h)
        for h in range(NSG):
            for ot in range(NOT):
                ysl = yt[ot][:, h * SGW : (h + 1) * SGW]
                sg = x_pool.tile([128, SGW], i8, name=f"sg{ot}_{h}", tag="x")
                if (ot, h) in DVE_SLABS:
                    if simple_tail:
                        # sg = (s >= mean) ; sg = 2*sg - 1
                        nc.vector.tensor_scalar(
                            sg[:], ysl, mv_t[ot][:, 0:1], None,
                            mybir.AluOpType.is_ge,
                        )
                        nc.vector.tensor_scalar(
                            sg[:], sg[:], 2.0, 1.0,
                            mybir.AluOpType.mult, mybir.AluOpType.subtract,
                        )
                    else:
                        # in-place: y' = y*A + B; sg = (y' >= 0); sg = 2*sg-1
                        nc.vector.tensor_scalar(
                            ysl, ysl, A_t[ot][:], B_t[ot][:],
                            mybir.AluOpType.mult, mybir.AluOpType.add,
                        )
                        nc.vector.tensor_scalar(
                            sg[:], ysl, 0.0, None, mybir.AluOpType.is_ge
                        )
                        nc.vector.tensor_scalar(
                            sg[:], sg[:], 2.0, 1.0,
                            mybir.AluOpType.mult, mybir.AluOpType.subtract,
                        )
                else:
                    nc.scalar.activation(
                        sg[:], ysl,
                        mybir.ActivationFunctionType.Sign,
                        bias=B_t[ot][:],
                        scale=A_t[ot][:],
                    )
                nc.sync.dma_start(
                    out_d.ap()[ot * 128 : (ot + 1) * 128, h * SGW : (h + 1) * SGW],
                    sg[:],
                )

    nc.compile()
    return nc


def prep_inputs(x, w, gamma, beta, n_cores=N_CORES, CH=512):
    """Host-side prep: transpose/chunk x (fp32), bw/coef shards per core."""
    B, I = x.shape
    O = w.shape[0]
    OSH = O // n_cores
    NT = I // 128
    NCH = B // CH

    # [I, B] -> [NT, NCH, 128, CH] -> pack i-tile pairs [NT//2, NCH, 128, 2CH]
    xt = np.ascontiguousarray(x.T)                # [I, B]
    xtp = xt.reshape(NT, 128, NCH, CH).transpose(0, 2, 1, 3)
    xtp = (
        xtp.reshape(NT // 2, 2, NCH, 128, CH)
        .transpose(0, 2, 3, 1, 4)
        .reshape(NT // 2, NCH, 128, 2 * CH)
    )
    xtp = np.ascontiguousarray(xtp)

    bw = np.sign(w).astype(np.float32)
    alpha = np.abs(w).mean(axis=1)                 # [O] f32
    p1 = alpha * alpha
    p2 = alpha * gamma
    p3 = alpha * alpha * gamma
    p4 = beta.astype(np.float32)

    in_maps = []
    for k in range(n_cores):
        sl = slice(k * OSH, (k + 1) * OSH)
        bwt = np.ascontiguousarray(bw[sl].T).reshape(NT, 128, OSH)
        NOT = OSH // 128

        def per_tile(vec):
            return np.ascontiguousarray(vec[sl].reshape(NOT, 128).T)  # [128, NOT]

        coef = np.concatenate(
            [per_tile(p1), per_tile(p2), per_tile(p3), per_tile(p4)], axis=1
        ).astype(np.float32)
        in_maps.append({
            "xtp": xtp,
            "bwt": bwt.astype(np.int8),
            "bwh": np.ascontiguousarray(bwt[:2]).astype(np.float32),
            "coef": coef,
        })
    return in_maps


_NC_CACHE = {}


def kernel(x, real_weight, gamma, beta):
    global LAST_RESULTS
    x = np.asarray(x, dtype=np.float32)
    w = np.asarray(real_weight, dtype=np.float32)
    gamma = np.asarray(gamma, dtype=np.float32)
    beta = np.asarray(beta, dtype=np.float32)
    B, I = x.shape
    O = w.shape[0]
    OSH = O // N_CORES
    CH = 512

    simple_tail = bool((gamma > 0).all() and (beta == 0).all())
    key = (B, I, OSH, CH, simple_tail)
    if key not in _NC_CACHE:
        _NC_CACHE[key] = build_nc(B, I, OSH, CH, simple_tail=simple_tail)
    nc = _NC_CACHE[key]

    in_maps = prep_inputs(x, w, gamma, beta, N_CORES, CH)
    trace = bool(int(os.environ.get("KERNEL_TRACE", "0")))
    res = run_bass_kernel_spmd(
        nc, in_maps, core_ids=list(range(N_CORES)), trace=trace
    )
    LAST_RESULTS = res

    out = np.empty((B, O), dtype=np.float32)
    for k in range(N_CORES):
        o = res.results[k]["out"]                  # [OSH, B] int8
        out[:, k * OSH : (k + 1) * OSH] = o.T.astype(np.float32)
    return out


# revision 4
# speedup vs baseline: 1.5908x; 1.1394x over previous
"""Trainium2 Bass kernel for nn_BinaryLinear (binary-weight linear + BatchNorm + sign).

Computation (reference):
    bw    = sign(W)                     # [O, I], entries in {-1, 0, +1}
    alpha = mean(|W|, axis=1)           # [O]
    y     = x @ (bw * alpha).T          # [B, O]
    out   = sign((y - mu_b) / sqrt(var_b + eps) * gamma + beta)   # batch stats

Strategy (8 NeuronCores, column-sharded):
  * Each core owns O/8 = 512 output columns; BN batch stats are then fully
    local to a core (full batch for its columns) -> no collectives.
  * alpha is factored out of the matmul: s = x @ bw.T is computed on the PE
    in float32r (fp32 inputs truncated to ~fp22 internally): a SINGLE
    matmul pass recovers ~13.2 mantissa bits of x (HW-measured), keeping
    sign flips well under the rel-err gate while HALVING PE work vs the
    bf16 hi/lo split (f32r streams 1 row/cycle at free-dim 512; measured
    227 ns/MM warm vs 216 for bf16).
  * Weights (+-1 exact in any dtype) are shipped as int8 and expanded to
    f32 by SWDGE cast-DMA, so the startup weight load costs 2 MB of HBM
    traffic instead of 8 MB, which matters because chunk 0 already needs
    the full 8 MB x stream.
  * Layout is transposed on host: s.T[o, b] so that o sits on SBUF
    partitions. BN stats are per-partition reductions along the free dim
    (BN_STATS/BN_AGGR); the final affine+sign is ScalarE/DVE slabs writing
    int8 (+-1), halving the tail output DMA.
  * SBUF: w 64K/part + y 128K/part leaves ~16K: x pairs and sign slabs
    share one 3-slot 4K pool ring (tag "x").
"""

import os
from contextlib import ExitStack

import ml_dtypes
import numpy as np

import concourse.bacc as bacc
import concourse.bass as bass
import concourse.mybir as mybir
import concourse.tile as tile
from concourse.bass_utils import run_bass_kernel_spmd

BF16 = ml_dtypes.bfloat16
BN_EPS = 1e-5

N_CORES = 8
B_FULL, IN_F, OUT_F = 8192, 4096, 4096

LAST_RESULTS = None  # BassKernelResults of the most recent device run


def build_nc(B, I, OSH, CH=512, xbufs=6, simple_tail=False):
    """Build + compile the per-core Bass program.

    B: batch (free dim of s.T), I: contraction, OSH: output columns per core,
    CH: batch chunk (<=512, PSUM bank / bn_stats limit). simple_tail may only
    be set when gamma > 0 and beta == 0 (sign(BN(y)) == sign(s - mean_s)).
    """
    NOT = OSH // 128          # o-tiles (PSUM partition groups)
    NT = I // 128             # i-tiles (contraction)
    NCH = B // CH             # batch chunks
    f32 = mybir.dt.float32
    f32r = mybir.dt.float32r
    bf16 = mybir.dt.bfloat16
    i8 = mybir.dt.int8

    nc = bacc.Bacc("TRN2", target_bir_lowering=False, debug=False)
    # x tiles packed in i-tile PAIRS: one DMA delivers fp32 chunks for two
    # consecutive i-tiles (512 KB).
    xtp_d = nc.dram_tensor(
        "xtp", [NT, NCH, 128, CH], f32r, kind="ExternalInput"
    )
    bwt_d = nc.dram_tensor("bwt", [NT, 128, OSH], i8, kind="ExternalInput")
    # first two i-tiles' weights also shipped raw f32: they ride the ACT
    # HWDGE ring (no cast on HWDGE) so the first matmul is not gated on
    # SWDGE spin-up.
    bwh_d = nc.dram_tensor("bwh", [2, 128, OSH], f32r, kind="ExternalInput")
    coef_d = nc.dram_tensor("coef", [128, 4 * NOT], f32, kind="ExternalInput")
    out_d = nc.dram_tensor("out", [OSH, B], i8, kind="ExternalOutput")
    SGW = min(B, 2048)        # sign-pass slab width (int8 slab = x slot size)
    NSG = B // SGW

    with tile.TileContext(nc) as tc, ExitStack() as ctx:
        bw_pool = ctx.enter_context(tc.tile_pool(name="bw", bufs=NT))
        x_pool = ctx.enter_context(tc.tile_pool(name="x", bufs=xbufs))
        y_pool = ctx.enter_context(tc.tile_pool(name="y", bufs=1))
        ps_pool = ctx.enter_context(
            tc.tile_pool(name="ps", bufs=8, space=bass.MemorySpace.PSUM)
        )
        st_pool = ctx.enter_context(tc.tile_pool(name="st", bufs=1))
        sm_pool = ctx.enter_context(tc.tile_pool(name="sm", bufs=NOT))

        # PE warm-up: the HAM clock gate holds the PE at 1.2 GHz until it has
        # been busy ~3.4us. Burn dummy matmuls during the initial DMA wait so
        # the real matmul stream starts at 2.4 GHz.
        wl = sm_pool.tile([128, 8], bf16)
        wr = sm_pool.tile([128, 8], bf16)
        nc.vector.memset(wl[:], 0.0)
        nc.vector.memset(wr[:], 0.0)
        wp = ps_pool.tile([128, CH], f32, name="wups", tag="ps")
        for _ in range(64):
            nc.tensor.matmul(wp[0:8, 0:8], wl[:], wr[:], start=True, stop=True)

        bw_tiles = [None] * NT
        for t in range(NT):
            bt = bw_pool.tile([128, OSH], f32r, name=f"bt{t}", tag="bt")
            if t < 2:
                nc.scalar.dma_start(bt[:], bwh_d.ap()[t])
            else:
                # int8 -> f32 cast during SWDGE DMA; values are +-1, exact.
                nc.gpsimd.dma_start(bt[:], bwt_d.ap()[t])
            bw_tiles[t] = bt

        ct = st_pool.tile([128, 4 * NOT], f32)
        nc.gpsimd.dma_start(ct[:], coef_d.ap())

        yt = [y_pool.tile([128, B], f32, name=f"yt{i}") for i in range(NOT)]
        stats = [st_pool.tile([128, 6 * NCH], f32, name=f"stats{i}") for i in range(NOT)]

        # Per-o-tile BN coefficients: with s-stats (mean_s, var_s) and host
        # precomputed p1=alpha^2, p2=alpha*gamma, p4=beta:
        #   inv = 1/sqrt(p1*var_s + eps);  A = p2*inv;  B = p4 - mean_s*A
        A_t, B_t, mv_t = [None] * NOT, [None] * NOT, [None] * NOT
        eps_t = sm_pool.tile([128, 1], f32)
        nc.vector.memset(eps_t[:], BN_EPS)

        def coef_chain(ot):
            mv = sm_pool.tile([128, 2], f32, name=f"mv{ot}", tag="mv")
            nc.vector.bn_aggr(mv[:], stats[ot][:])
            p1 = ct[:, ot : ot + 1]
            p2 = ct[:, NOT + ot : NOT + ot + 1]
            p4 = ct[:, 3 * NOT + ot : 3 * NOT + ot + 1]
            v = sm_pool.tile([128, 1], f32, name=f"v{ot}", tag="v")
            nc.vector.tensor_mul(v[:], mv[:, 1:2], p1)
            sd = sm_pool.tile([128, 1], f32, name=f"sd{ot}", tag="sd")
            nc.scalar.activation(
                sd[:], v[:], mybir.ActivationFunctionType.Sqrt, bias=eps_t[:]
            )
            inv = sm_pool.tile([128, 1], f32, name=f"inv{ot}", tag="inv")
            nc.vector.reciprocal(inv[:], sd[:])
            Ac = sm_pool.tile([128, 1], f32, name=f"Ac{ot}", tag="Ac")
            nc.vector.tensor_mul(Ac[:], p2, inv[:])
            mB = sm_pool.tile([128, 1], f32, name=f"mB{ot}", tag="mB")
            nc.vector.tensor_mul(mB[:], mv[:, 0:1], Ac[:])
            Bc = sm_pool.tile([128, 1], f32, name=f"Bc{ot}", tag="Bc")
            nc.vector.tensor_sub(Bc[:], p4, mB[:])
            A_t[ot], B_t[ot], mv_t[ot] = Ac, Bc, mv

        for c in range(NCH):
            if c == 1:
                # Preload the tail ACT LUTs (Sqrt, Sign) once the stream is
                # rolling: no ACT_TABLE_LOAD on the critical tail, and no
                # delay to the startup DMA issues on the ACT ring.
                wt = sm_pool.tile([128, 1], f32)
                nc.vector.memset(wt[:], 1.0)
                wt2 = sm_pool.tile([128, 1], f32)
                nc.scalar.activation(
                    wt2[:], wt[:], mybir.ActivationFunctionType.Sqrt
                )
                nc.scalar.activation(
                    wt2[:], wt[:], mybir.ActivationFunctionType.Sign
                )
            ps = [ps_pool.tile([128, CH], f32, name=f"ps{c}_{i}", tag="ps") for i in range(NOT)]
            for t in range(NT):
                xt = x_pool.tile([128, CH], f32r, tag="x")
                # single-tile DMAs on alternating HWDGE rings (SP / ACT):
                # finer arrival granularity + 6-deep ring absorbs the DMA
                # transfer+completion-semaphore latency chain
                dma_eng = nc.sync if t % 2 == 0 else nc.scalar
                dma_eng.dma_start(xt[:], xtp_d.ap()[t, c])
                for ot in range(NOT):
                    lhsT = bw_tiles[t][:, ot * 128 : (ot + 1) * 128]
                    nc.tensor.matmul(
                        ps[ot][:], lhsT, xt[:, 0:CH],
                        start=(t == 0), stop=(t == NT - 1),
                    )
            for ot in range(NOT):
                ysl = yt[ot][:, c * CH : (c + 1) * CH]
                # stats read PSUM directly so the tail-critical chain does not
                # wait on the ACT evacuation copy
                nc.vector.bn_stats(stats[ot][:, c * 6 : (c + 1) * 6], ps[ot][:])
                if c == NCH - 1:
                    # emit the coefficient chain right after this o-tile's
                    # final stats so the first sign slab starts ASAP; the
                    # final evacuation rides ACT so DVE reaches its sign
                    # slabs sooner
                    coef_chain(ot)
                    nc.scalar.copy(ysl, ps[ot][:])
                else:
                    nc.vector.tensor_copy(ysl, ps[ot][:])

        # Final affine+sign, split between ScalarE (one ACTIVATE(Sign) per
        # slab, 1 op) and DVE (2 ops simple / 3 ops full) so the serial tail
        # drains on two engines at once. Slab outputs are int8 (+-1), which
        # halves the tail DMA-out bytes.
        n_slabs = NOT * NSG
        step = 3 if simple_tail else 4
        dve_ks = set(range(1, n_slabs, step)) if n_slabs >= 6 else set()
        DVE_SLABS = {divmod(k, NOT)[::-1] for k in dve_ks}  # k=h*NOT+ot -> (ot,h)
        for h in range(NSG):
            for ot in range(NOT):
                ysl = yt[ot][:, h * SGW : (h + 1) * SGW]
                sg = x_pool.tile([128, SGW], i8, name=f"sg{ot}_{h}", tag="x")
                if (ot, h) in DVE_SLABS:
                    if simple_tail:
                        # sg = (s >= mean) ; sg = 2*sg - 1
                        nc.vector.tensor_scalar(
                            sg[:], ysl, mv_t[ot][:, 0:1], None,
                            mybir.AluOpType.is_ge,
                        )
                        nc.vector.tensor_scalar(
                            sg[:], sg[:], 2.0, 1.0,
                            mybir.AluOpType.mult, mybir.AluOpType.subtract,
                        )
                    else:
                        # in-place: y' = y*A + B; sg = (y' >= 0); sg = 2*sg-1
                        nc.vector.tensor_scalar(
                            ysl, ysl, A_t[ot][:], B_t[ot][:],
                            mybir.AluOpType.mult, mybir.AluOpType.add,
                        )
                        nc.vector.tensor_scalar(
                            sg[:], ysl, 0.0, None, mybir.AluOpType.is_ge
                        )
                        nc.vector.tensor_scalar(
                            sg[:], sg[:], 2.0, 1.0,
                            mybir.AluOpType.mult, mybir.AluOpType.subtract,
                        )
                else:
                    nc.scalar.activation(
                        sg[:], ysl,
                        mybir.ActivationFunctionType.Sign,
                        bias=B_t[ot][:],
                        scale=A_t[ot][:],
                    )
                nc.sync.dma_start(
                    out_d.ap()[ot * 128 : (ot + 1) * 128, h * SGW : (h + 1) * SGW],
                    sg[:],
                )

    nc.compile()
    return nc


def prep_inputs(x, w, gamma, beta, n_cores=N_CORES, CH=512):
    """Host-side prep: transpose/chunk x (fp32), bw/coef shards per core."""
    B, I = x.shape
    O = w.shape[0]
    OSH = O // n_cores
    NT = I // 128
    NCH = B // CH

    # [I, B] -> [NT, NCH, 128, CH]
    xt = np.ascontiguousarray(x.T)                # [I, B]
    xtp = np.ascontiguousarray(
        xt.reshape(NT, 128, NCH, CH).transpose(0, 2, 1, 3)
    )

    bw = np.sign(w).astype(np.float32)
    alpha = np.abs(w).mean(axis=1)                 # [O] f32
    p1 = alpha * alpha
    p2 = alpha * gamma
    p3 = alpha * alpha * gamma
    p4 = beta.astype(np.float32)

    in_maps = []
    for k in range(n_cores):
        sl = slice(k * OSH, (k + 1) * OSH)
        bwt = np.ascontiguousarray(bw[sl].T).reshape(NT, 128, OSH)
        NOT = OSH // 128

        def per_tile(vec):
            return np.ascontiguousarray(vec[sl].reshape(NOT, 128).T)  # [128, NOT]

        coef = np.concatenate(
            [per_tile(p1), per_tile(p2), per_tile(p3), per_tile(p4)], axis=1
        ).astype(np.float32)
        in_maps.append({
            "xtp": xtp,
            "bwt": bwt.astype(np.int8),
            "bwh": np.ascontiguousarray(bwt[:2]).astype(np.float32),
            "coef": coef,
        })
    return in_maps


_NC_CACHE = {}


def kernel(x, real_weight, gamma, beta):
    global LAST_RESULTS
    x = np.asarray(x, dtype=np.float32)
    w = np.asarray(real_weight, dtype=np.float32)
    gamma = np.asarray(gamma, dtype=np.float32)
    beta = np.asarray(beta, dtype=np.float32)
    B, I = x.shape
    O = w.shape[0]
    OSH = O // N_CORES
    CH = 512

    simple_tail = bool((gamma > 0).all() and (beta == 0).all())
    key = (B, I, OSH, CH, simple_tail)
    if key not in _NC_CACHE:
        _NC_CACHE[key] = build_nc(B, I, OSH, CH, simple_tail=simple_tail)
    nc = _NC_CACHE[key]

    in_maps = prep_inputs(x, w, gamma, beta, N_CORES, CH)
    trace = bool(int(os.environ.get("KERNEL_TRACE", "0")))
    res = run_bass_kernel_spmd(
        nc, in_maps, core_ids=list(range(N_CORES)), trace=trace
    )
    LAST_RESULTS = res

    out = np.empty((B, O), dtype=np.float32)
    for k in range(N_CORES):
        o = res.results[k]["out"]                  # [OSH, B] int8
        out[:, k * OSH : (k + 1) * OSH] = o.T.astype(np.float32)
    return out
